# revision 12
# baseline (speedup 1.0000x reference)
"""Block-causal multi-head self-attention for TRN2, sharded over 8 NeuronCores.

Problem (hardcoded): B=2, T=2048 (512 frames x 4 animals), C=512, H=8 heads,
D=64. Block-causal mask = kron(tril(frames), ones(4,4)); key mask is all-ones
per the input spec (a numpy fallback handles the general case).

Sharding: core i handles batch b = i//4 and heads (2*(i%4), 2*(i%4)+1).
Wq/Wk/Wv are column-sharded (128 cols per core), Wp row-sharded (128 rows per
core). Each core emits a partial [T, C] output projection; the host sums the
4 partials per batch and adds bp.

On-core algorithm (bf16 matmul operands, fp32 PSUM accumulation):
  xT = transpose(x)  (PE transposes; x streamed in bf16)
  qT/kT/vT = W.T @ xT  (bias folded into the PSUM->SBUF copy; softmax scale
  folded into Wq on host)
  per head h, per 256-wide query chunk qc:
    S^T[j,i] = kT^T(128-j-block) @ qT(chunk)    (K=64, blocks j<=diag)
    diagonal tri-mask added via a rank-64 indicator matmul (-1e9 offside)
    P^T = exp(S^T)  (ACT, PSUM->bf16 SBUF, groups of 4 j-blocks)
    O^T_aug[65,256] += V_aug(block).T @ P^T     (V col 64 = ones -> row 64 = l)
  l column per 128-row t-tile via a [65,1] selector matmul; rl = 1/l
  partial = (O^T_h[:64].T @ Wp_h) * rl_h summed over the 2 heads (ACT/DVE)
The projection tail for chunk qc is emitted after the attention of chunk
qc+1 so the in-order PE never stalls on the DVE copies of O^T.
"""

import math

import numpy as np
import ml_dtypes

import concourse.bass as bass
import concourse.bacc as bacc
import concourse.tile as tile
from concourse import mybir
from concourse.bass_utils import run_bass_kernel_spmd

B, T, C, H, D = 2, 2048, 512, 8, 64
NF, NA = 512, 4
NCORES = 8
HPC = 2            # heads per core
CPB = 4            # cores per batch
SCALE = 1.0 / math.sqrt(D)
NEG = -1.0e9
IC = 256           # query-chunk width
NQC = T // IC      # 8
GRP = 4            # j-blocks (128 keys each) per exp group

F32 = mybir.dt.float32
BF = mybir.dt.bfloat16
BF_NP = ml_dtypes.bfloat16


def _emit(ctx, tc, out_d, in_d):
    nc = tc.nc
    ts = bass.ts
    Exp = mybir.ActivationFunctionType.Exp
    Ident = mybir.ActivationFunctionType.Identity
    mult, add = mybir.AluOpType.mult, mybir.AluOpType.add

    const = ctx.enter_context(tc.tile_pool(name="const", bufs=1))
    big = ctx.enter_context(tc.tile_pool(name="big", bufs=1))

    xT = big.tile([128, 4 * T], BF, tag="xT")

    # Phase A: hardware XBAR DMA-transpose x[t, c] -> xT[c, t]; 16 transfers
    # of [512 t, 128 c] so phase B's per-t5 matmuls start as chunks land.
    for t5 in range(4):
        for cb in range(4):
            nc.sync.dma_start_transpose(
                out=xT[:, cb * T + t5 * 512 : cb * T + (t5 + 1) * 512],
                in_=in_d["x"][t5 * 512 : (t5 + 1) * 512, ts(cb, 128)],
            )

    # Remaining constants (DMAs overlap phase A compute).
    sel = const.tile([65, 1], BF)
    nc.sync.dma_start(out=sel[:], in_=in_d["sel65"][:])
    identh = const.tile([128, 64], BF)
    nc.sync.dma_start(out=identh[:], in_=in_d["identh"][:])

    w_sb, b_sb = {}, {}
    for nm in ("wq", "wk", "wv"):
        w = const.tile([128, 512], BF, tag=f"w_{nm}")
        for cb in range(4):
            nc.sync.dma_start(out=w[:, ts(cb, 128)], in_=in_d[nm][ts(cb, 128), :])
        w_sb[nm] = w
        bt = const.tile([128, 1], F32, tag=f"b_{nm}")
        nc.sync.dma_start(out=bt[:], in_=in_d["b" + nm[1]][:])
        b_sb[nm] = bt
    wps = []
    for h in range(HPC):
        wph = const.tile([64, 512], BF, tag=f"wp{h}", name=f"wp{h}")
        nc.sync.dma_start(out=wph[:], in_=in_d["wp"][ts(h, 64), :])
        wps.append(wph)

    qA = [big.tile([128, T], BF, tag=f"qA{h}", name=f"qA{h}") for h in range(HPC)]
    kA = [big.tile([128, T], BF, tag=f"kA{h}", name=f"kA{h}") for h in range(HPC)]
    vT = big.tile([128, T], BF, tag="vT")
    # mask rows: head0 tiles carry them in partitions 64:128, head1 in 0:64
    nc.sync.dma_start(out=kA[0][64:128, :], in_=in_d["mask_k"][:])
    nc.sync.dma_start(out=kA[1][0:64, :], in_=in_d["mask_k"][:])
    nc.sync.dma_start(out=qA[0][64:128, :], in_=in_d["mask_q"][:])
    nc.sync.dma_start(out=qA[1][0:64, :], in_=in_d["mask_q"][:])
    Vb = big.tile([128, HPC * 16 * 65], BF, tag="Vb")
    OTs = [big.tile([65, T], BF, tag=f"OT{h}", name=f"OT{h}") for h in range(HPC)]
    rl = big.tile([128, HPC * 16], F32, tag="rl")

    # Phase B: projections -> qT/kT/vT in [head_ch, t] layout; bias folded
    # into the PSUM->SBUF copy (ACT per-partition bias / DVE tensor_scalar).
    with tc.tile_pool(name="psproj", bufs=2, space="PSUM") as psp:
        for nm, dst in (("wq", qA), ("wk", kA), ("wv", vT)):
            for t5 in range(4):
                ps = psp.tile([128, 512], F32)
                for cb in range(4):
                    nc.tensor.matmul(
                        ps[:],
                        w_sb[nm][:, ts(cb, 128)],
                        xT[:, cb * T + t5 * 512 : cb * T + (t5 + 1) * 512],
                        start=(cb == 0),
                        stop=(cb == 3),
                    )
                if nm == "wv":
                    if t5 % 2 == 0:
                        nc.scalar.activation(dst[:, ts(t5, 512)], ps[:], Ident,
                                             bias=b_sb[nm][:])
                    else:
                        nc.vector.tensor_scalar_add(dst[:, ts(t5, 512)], ps[:],
                                                    b_sb[nm][:])
                else:
                    nc.scalar.activation(dst[0][0:64, ts(t5, 512)], ps[0:64, :],
                                         Ident, bias=b_sb[nm][0:64, :])
                    nc.vector.tensor_scalar_add(dst[1][64:128, ts(t5, 512)],
                                                ps[64:128, :],
                                                b_sb[nm][64:128, :])

    # Phase C: V_aug blocks [128 j, 65] per (head, j-block); col 64 = ones.
    with tc.tile_pool(name="psv", bufs=4, space="PSUM") as psv:
        for h in range(HPC):
            for jb in range(16):
                pv = psv.tile([128, 64], BF)
                nc.tensor.transpose(
                    pv[:], vT[ts(h, 64), ts(jb, 128)], identh[ts(h, 64), :]
                )
                base = (h * 16 + jb) * 65
                nc.vector.tensor_copy(Vb[:, base : base + 64], pv[:])
                nc.vector.memset(Vb[:, base + 64 : base + 65], 1.0)

    # Phase D/E: attention + projection, software-pipelined per query chunk
    # (the projection tail of chunk qc is emitted during chunk qc+1).
    with tc.tile_pool(name="pss", bufs=2, space="PSUM") as pss, \
         tc.tile_pool(name="psot", bufs=1, space="PSUM") as psot, \
         tc.tile_pool(name="psl", bufs=1, space="PSUM") as pslp, \
         tc.tile_pool(name="pspr", bufs=2, space="PSUM") as pspr, \
         tc.tile_pool(name="ptp", bufs=3) as ptp, \
         tc.tile_pool(name="comb", bufs=4) as comb:

        def attention(qc):
            nbj = 2 * (qc + 1)
            for h in range(HPC):
                hs = slice(0, 64) if h == 0 else slice(64, 128)
                ot = psot.tile([65, IC], F32, tag="ot")
                for g0 in range(0, nbj, GRP):
                    ng = min(GRP, nbj - g0)
                    ps = pss.tile([128, GRP * IC], F32)
                    for k in range(ng):
                        jb = g0 + k
                        sl = ps[:, ts(k, IC)]
                        if jb >= 2 * qc:
                            nc.tensor.matmul(
                                sl,
                                kA[h][:, ts(jb, 128)],
                                qA[h][:, ts(qc, IC)],
                                start=True, stop=True,
                            )
                        else:
                            nc.tensor.matmul(
                                sl,
                                kA[h][hs, ts(jb, 128)],
                                qA[h][hs, ts(qc, IC)],
                                start=True, stop=True,
                            )
                    ptt = ptp.tile([128, GRP * IC], BF)
                    nc.scalar.activation(ptt[:, 0 : ng * IC], ps[:, 0 : ng * IC], Exp)
                    for k in range(ng):
                        jb = g0 + k
                        vbase = (h * 16 + jb) * 65
                        nc.tensor.matmul(
                            ot[:],
                            Vb[:, vbase : vbase + 65],
                            ptt[:, ts(k, IC)],
                            start=(jb == 0),
                            stop=(jb == nbj - 1),
                        )
                nc.vector.tensor_copy(OTs[h][:, ts(qc, IC)], ot[:])

        def proj(qc):
            psl = pslp.tile([128, 4], F32, tag="sel")
            for i2, t2 in enumerate((2 * qc, 2 * qc + 1)):
                for h in range(HPC):
                    col = h * 16 + t2
                    nc.tensor.matmul(
                        psl[:, i2 * 2 + h : i2 * 2 + h + 1],
                        OTs[h][:, ts(t2, 128)],
                        sel[:],
                        start=True,
                        stop=True,
                    )
                    nc.vector.reciprocal(rl[:, col : col + 1],
                                         psl[:, i2 * 2 + h : i2 * 2 + h + 1])
                pr0 = pspr.tile([128, 512], F32, tag="pr")
                nc.tensor.matmul(
                    pr0[:], OTs[0][0:64, ts(t2, 128)], wps[0][:],
                    start=True, stop=True,
                )
                pr1 = pspr.tile([128, 512], F32, tag="pr")
                nc.tensor.matmul(
                    pr1[:], OTs[1][0:64, ts(t2, 128)], wps[1][:],
                    start=True, stop=True,
                )
                tmp = comb.tile([128, 512], F32, tag="tmp")
                nc.vector.tensor_scalar_mul(tmp[:], pr0[:], rl[:, t2 : t2 + 1])
                ob = comb.tile([128, 512], F32, tag="ob")
                nc.vector.scalar_tensor_tensor(
                    ob[:], pr1[:], rl[:, 16 + t2 : 16 + t2 + 1], tmp[:], mult, add
                )
                nc.sync.dma_start(out=out_d[ts(t2, 128), :], in_=ob[:])

        for qc in range(NQC):
            attention(qc)
            if qc > 0:
                proj(qc - 1)
        proj(NQC - 1)


_PROGRAM_CACHE = {}
TRACE = False
_LAST = {}


def _build_program():
    key = ("prog", GRP)
    if key in _PROGRAM_CACHE:
        return _PROGRAM_CACHE[key]
    from contextlib import ExitStack

    nc = bacc.Bacc(trn_type="TRN2", target_bir_lowering=False, debug=False,
                   num_devices=NCORES)
    in_d = {
        "x": nc.dram_tensor("x", [T, C], BF, kind="ExternalInput").ap(),
        "wq": nc.dram_tensor("wq", [C, 128], BF, kind="ExternalInput").ap(),
        "wk": nc.dram_tensor("wk", [C, 128], BF, kind="ExternalInput").ap(),
        "wv": nc.dram_tensor("wv", [C, 128], BF, kind="ExternalInput").ap(),
        "bq": nc.dram_tensor("bq", [128, 1], F32, kind="ExternalInput").ap(),
        "bk": nc.dram_tensor("bk", [128, 1], F32, kind="ExternalInput").ap(),
        "bv": nc.dram_tensor("bv", [128, 1], F32, kind="ExternalInput").ap(),
        "wp": nc.dram_tensor("wp", [128, C], BF, kind="ExternalInput").ap(),
        "identh": nc.dram_tensor("identh", [128, 64], BF, kind="ExternalInput").ap(),
        "mask_k": nc.dram_tensor("mask_k", [64, T], BF, kind="ExternalInput").ap(),
        "mask_q": nc.dram_tensor("mask_q", [64, T], BF, kind="ExternalInput").ap(),
        "sel65": nc.dram_tensor("sel65", [65, 1], BF, kind="ExternalInput").ap(),
    }
    out_d = nc.dram_tensor("out", [T, C], F32, kind="ExternalOutput").ap()
    with tile.TileContext(nc) as tc:
        with ExitStack() as ctx:
            _emit(ctx, tc, out_d, in_d)
    nc.compile()
    _PROGRAM_CACHE[key] = nc
    return nc


def _consts():
    f = np.float32
    ident = np.eye(128, dtype=f)
    rr = np.arange(64)
    jj = np.arange(T)
    # mask_k[r, j] = 1 where r == 32*(jb%2) + (j%128)//4 (jb = j//128)
    mk = (rr[:, None] == 32 * ((jj[None, :] // 128) % 2) + (jj[None, :] % 128) // NA)
    mask_k = mk.astype(f)
    # mask_q[r, i] = NEG where (i%256)//4 < r
    mask_q = np.where((jj[None, :] % IC) // NA < rr[:, None], f(NEG), f(0.0)).astype(f)
    sel = np.zeros((65, 1), dtype=f)
    sel[64, 0] = 1.0
    identh = np.vstack([np.eye(64, dtype=f), np.eye(64, dtype=f)])
    return ident, mask_k, mask_q, sel, identh


def _numpy_reference(x, mask, Wq, bq, Wk, bk, Wv, bv, Wp, bp):
    b, t, c = x.shape
    h, d = H, c // H
    scale = 1.0 / math.sqrt(d)
    tril = np.tril(np.ones((NF, NF), dtype=np.float32))
    block = np.kron(tril, np.ones((NA, NA), dtype=np.float32))

    def heads(w, bias):
        return (x @ w + bias).reshape(b, t, h, d).transpose(0, 2, 1, 3)

    q, k, v = heads(Wq, bq), heads(Wk, bk), heads(Wv, bv)
    att = np.einsum("bhqd,bhkd->bhqk", q, k) * scale
    allowed = block[None, None] * mask[:, None, None, :].astype(np.float32)
    att = np.where(allowed == 0, -np.inf, att)
    att = att - att.max(axis=-1, keepdims=True)
    att = np.exp(att)
    att = att / att.sum(axis=-1, keepdims=True)
    y = np.einsum("bhqk,bhkd->bhqd", att, v)
    y = y.transpose(0, 2, 1, 3).reshape(b, t, c)
    return (y @ Wp + bp).astype(np.float32)


def _bf(a):
    return np.ascontiguousarray(a).astype(BF_NP)


def kernel(**inputs):
    x = np.asarray(inputs["x"], dtype=np.float32)
    mask = np.asarray(inputs["mask"])
    Wq = np.asarray(inputs["Wq"], dtype=np.float32)
    bq = np.asarray(inputs["bq"], dtype=np.float32)
    Wk = np.asarray(inputs["Wk"], dtype=np.float32)
    bk = np.asarray(inputs["bk"], dtype=np.float32)
    Wv = np.asarray(inputs["Wv"], dtype=np.float32)
    bv = np.asarray(inputs["bv"], dtype=np.float32)
    Wp = np.asarray(inputs["Wp"], dtype=np.float32)
    bp = np.asarray(inputs["bp"], dtype=np.float32)

    if not np.all(np.asarray(mask) == 1):
        return _numpy_reference(x, mask, Wq, bq, Wk, bk, Wv, bv, Wp, bp)

    nc = _build_program()
    ident, mask_k, mask_q, sel, identh = _consts()
    in_maps = []
    for core in range(NCORES):
        b = core // CPB
        hp = core % CPB
        cs = slice(hp * 128, (hp + 1) * 128)
        in_maps.append({
            "x": _bf(x[b]),
            "wq": _bf(Wq[:, cs] * np.float32(SCALE)),
            "wk": _bf(Wk[:, cs]),
            "wv": _bf(Wv[:, cs]),
            "bq": (bq[cs] * np.float32(SCALE)).reshape(128, 1).copy(),
            "bk": bk[cs].reshape(128, 1).copy(),
            "bv": bv[cs].reshape(128, 1).copy(),
            "wp": _bf(Wp[cs, :]),
            "identh": _bf(identh),
            "mask_k": _bf(mask_k),
            "mask_q": _bf(mask_q),
            "sel65": _bf(sel),
        })
    rr = run_bass_kernel_spmd(
        nc, in_maps, list(range(NCORES)), trace=TRACE,
        tmpdir=_LAST.get("tmpdir"),
    )
    _LAST["results"] = rr
    res = rr.results
    out = np.zeros((B, T, C), dtype=np.float32)
    for b in range(B):
        acc = res[b * CPB]["out"].astype(np.float32)
        for j in range(1, CPB):
            acc = acc + res[b * CPB + j]["out"]
        out[b] = acc + bp[None, :]
    return out


# revision 14
# speedup vs baseline: 1.1330x; 1.1330x over previous
"""Block-causal multi-head self-attention for TRN2, sharded over 8 NeuronCores.

Problem (hardcoded): B=2, T=2048 (512 frames x 4 animals), C=512, H=8 heads,
D=64. Block-causal mask = kron(tril(frames), ones(4,4)); key mask is all-ones
per the input spec (a numpy fallback handles the general case).

Sharding: core i handles batch b = i//4 and heads (2*(i%4), 2*(i%4)+1).
Wq/Wk/Wv are column-sharded (128 cols per core), Wp row-sharded (128 rows per
core). Each core emits a partial [T, C] output projection; the host sums the
4 partials per batch and adds bp.

On-core algorithm (bf16 matmul operands, fp32 PSUM accumulation):
  A: x streamed in 16 row-tiles (DMA issue alternates the two HWDGE rings);
     PE-transposed into xT[c, t]. Weights arrive as one packed blob DMA.
  B: qT/kT/vT = W.T @ xT (bias folded into the PSUM->SBUF copy; softmax
     scale folded into Wq on host).
  C: V_aug blocks [128 j, 65] per (head, j-block); col 64 = ones.
  D: per 128-query chunk tt (j-blocks 0..tt; only jb==tt needs the intra-
     block tri-mask, added via rank-64 indicator rows inside the S matmul):
       S^T[j,i] groups of <=4 j-blocks x 2 heads -> one [128,<=1024] PSUM
       P^T = exp(S^T)  (ACT, PSUM -> bf16 SBUF)
       O[i, d_aug] += P^T(block).T @ V_aug(block)   (N=65 rows per matmul;
         col 64 accumulates l)
     rl = 1/O[:,64] (DVE); OS[t, (h d)] = O[:,:64]*rl (DVE, bf16)
     one PE transpose of OS[t-tile] -> stacked OT[(h d), t]; single K=128
     matmul (OT, Wp-stacked) -> partial out tile; DVE copy; DMA out.
     The projection tail of chunk tt is emitted during chunk tt+1.
"""

import math

import numpy as np
import ml_dtypes

import concourse.bass as bass
import concourse.bacc as bacc
import concourse.tile as tile
from concourse import mybir
from concourse.bass_utils import run_bass_kernel_spmd

B, T, C, H, D = 2, 2048, 512, 8, 64
NF, NA = 512, 4
NCORES = 8
HPC = 2            # heads per core
CPB = 4            # cores per batch
SCALE = 1.0 / math.sqrt(D)
NEG = -1.0e9
IC = 128           # query-chunk width
NQC = T // IC      # 16
GRP = 4            # j-blocks per exp group (per head)

F32 = mybir.dt.float32
BF = mybir.dt.bfloat16
BF_NP = ml_dtypes.bfloat16

# packed weight blob column offsets (bf16, [128, BLOB_W])
_OFF_WQ, _OFF_WK, _OFF_WV = 0, 512, 1024
_OFF_WP, _OFF_IDH = 1536, 2048
BLOB_W = 2112


def _emit(ctx, tc, out_d, in_d):
    nc = tc.nc
    ts = bass.ts
    Exp = mybir.ActivationFunctionType.Exp
    Ident = mybir.ActivationFunctionType.Identity

    const = ctx.enter_context(tc.tile_pool(name="const", bufs=1))
    big = ctx.enter_context(tc.tile_pool(name="big", bufs=1))

    # ident first on the SP ring (phase A needs it); blob + biases on the ACT
    # ring so descriptor generation for x never queues behind them.
    ident = const.tile([128, 128], BF)
    nc.sync.dma_start(out=ident[:], in_=in_d["ident"][:])
    blob = const.tile([128, BLOB_W], BF)
    nc.scalar.dma_start(out=blob[:], in_=in_d["blob"][:])
    bias = const.tile([128, 3], F32)
    nc.scalar.dma_start(out=bias[:], in_=in_d["bias"][:])

    w_sb = {"wq": blob[:, _OFF_WQ : _OFF_WQ + 512],
            "wk": blob[:, _OFF_WK : _OFF_WK + 512],
            "wv": blob[:, _OFF_WV : _OFF_WV + 512]}
    b_sb = {"wq": bias[:, 0:1], "wk": bias[:, 1:2], "wv": bias[:, 2:3]}
    wp_sb = blob[:, _OFF_WP : _OFF_WP + 512]
    identh = blob[:, _OFF_IDH : _OFF_IDH + 64]

    xT = big.tile([128, 4 * T], BF, tag="xT")

    # Phase A: stream x in, transpose to xT[c, t] (4 c-chunks along free dim).
    with tc.tile_pool(name="xin", bufs=4) as xin, \
         tc.tile_pool(name="pst", bufs=4, space="PSUM") as pst:
        for tb in range(16):
            xt_ = xin.tile([128, 512], BF)
            eng = nc.sync if tb % 2 == 0 else nc.scalar
            eng.dma_start(out=xt_[:], in_=in_d["x"][ts(tb, 128), :])
            for cb in range(4):
                pt = pst.tile([128, 128], BF)
                nc.tensor.transpose(pt[:], xt_[:, ts(cb, 128)], ident[:])
                dst = xT[:, cb * T + tb * 128 : cb * T + (tb + 1) * 128]
                if cb % 2 == 0:
                    nc.vector.tensor_copy(dst, pt[:])
                else:
                    nc.scalar.activation(dst, pt[:], Ident)

    qA = [big.tile([128, T], BF, tag=f"qA{h}", name=f"qA{h}") for h in range(HPC)]
    kA = [big.tile([128, T], BF, tag=f"kA{h}", name=f"kA{h}") for h in range(HPC)]
    vT = big.tile([128, T], BF, tag="vT")
    # mask rows: head0 tiles carry them in partitions 64:128, head1 in 0:64
    nc.sync.dma_start(out=kA[0][64:128, :], in_=in_d["mask_k"][:])
    nc.scalar.dma_start(out=kA[1][0:64, :], in_=in_d["mask_k"][:])
    nc.sync.dma_start(out=qA[0][64:128, :], in_=in_d["mask_q"][:])
    nc.scalar.dma_start(out=qA[1][0:64, :], in_=in_d["mask_q"][:])
    Vb = big.tile([128, HPC * 16 * 65], BF, tag="Vb")
    OSb = big.tile([128, T], BF, tag="OSb")
    rl = big.tile([128, HPC * NQC], F32, tag="rl")

    # Phase B: projections -> qT/kT/vT in [head_ch, t] layout; bias folded
    # into the PSUM->SBUF copy (ACT per-partition bias / DVE tensor_scalar).
    with tc.tile_pool(name="psproj", bufs=2, space="PSUM") as psp:
        for nm, dst in (("wq", qA), ("wk", kA), ("wv", vT)):
            for t5 in range(4):
                ps = psp.tile([128, 512], F32)
                for cb in range(4):
                    nc.tensor.matmul(
                        ps[:],
                        w_sb[nm][:, ts(cb, 128)],
                        xT[:, cb * T + t5 * 512 : cb * T + (t5 + 1) * 512],
                        start=(cb == 0),
                        stop=(cb == 3),
                    )
                if nm == "wv":
                    if t5 % 2 == 0:
                        nc.scalar.activation(dst[:, ts(t5, 512)], ps[:], Ident,
                                             bias=b_sb[nm])
                    else:
                        nc.vector.tensor_scalar_add(dst[:, ts(t5, 512)], ps[:],
                                                    b_sb[nm])
                else:
                    nc.scalar.activation(dst[0][0:64, ts(t5, 512)], ps[0:64, :],
                                         Ident, bias=b_sb[nm][0:64, :])
                    nc.vector.tensor_scalar_add(dst[1][64:128, ts(t5, 512)],
                                                ps[64:128, :],
                                                b_sb[nm][64:128, :])

    # Phase C: V_aug blocks [128 j, 65] per (head, j-block); col 64 = ones.
    with tc.tile_pool(name="psv", bufs=4, space="PSUM") as psv:
        for h in range(HPC):
            for jb in range(16):
                pv = psv.tile([128, 64], BF)
                nc.tensor.transpose(
                    pv[:], vT[ts(h, 64), ts(jb, 128)], identh[ts(h, 64), :]
                )
                base = (h * 16 + jb) * 65
                nc.vector.tensor_copy(Vb[:, base : base + 64], pv[:])
                nc.vector.memset(Vb[:, base + 64 : base + 65], 1.0)

    # Phase D: attention (128-query chunks) + projection, software-pipelined.
    with tc.tile_pool(name="pss", bufs=2, space="PSUM") as pss, \
         tc.tile_pool(name="pso", bufs=1, space="PSUM") as pso, \
         tc.tile_pool(name="pstE", bufs=1, space="PSUM") as pstE, \
         tc.tile_pool(name="pspr", bufs=1, space="PSUM") as pspr, \
         tc.tile_pool(name="ptp", bufs=3) as ptp, \
         tc.tile_pool(name="otbp", bufs=2) as otbp, \
         tc.tile_pool(name="comb", bufs=2) as comb:

        def attention(tt):
            nbj = tt + 1
            o = [pso.tile([128, 65], F32, tag=f"o{h}", name=f"o{h}")
                 for h in range(HPC)]
            for g0 in range(0, nbj, GRP):
                ng = min(GRP, nbj - g0)
                ps = pss.tile([128, 2 * GRP * 128], F32)
                for h in range(HPC):
                    hs = slice(0, 64) if h == 0 else slice(64, 128)
                    for k in range(ng):
                        jb = g0 + k
                        sl = ps[:, (h * ng + k) * 128 : (h * ng + k + 1) * 128]
                        if jb == tt:
                            nc.tensor.matmul(
                                sl, kA[h][:, ts(jb, 128)], qA[h][:, ts(tt, IC)],
                                start=True, stop=True,
                            )
                        else:
                            nc.tensor.matmul(
                                sl, kA[h][hs, ts(jb, 128)], qA[h][hs, ts(tt, IC)],
                                start=True, stop=True,
                            )
                ptt = ptp.tile([128, 2 * GRP * 128], BF)
                nc.scalar.activation(ptt[:, 0 : 2 * ng * 128],
                                     ps[:, 0 : 2 * ng * 128], Exp)
                for h in range(HPC):
                    for k in range(ng):
                        jb = g0 + k
                        vbase = (h * 16 + jb) * 65
                        nc.tensor.matmul(
                            o[h][:],
                            ptt[:, (h * ng + k) * 128 : (h * ng + k + 1) * 128],
                            Vb[:, vbase : vbase + 65],
                            start=(jb == 0),
                            stop=(jb == nbj - 1),
                        )
            for h in range(HPC):
                col = h * NQC + tt
                nc.vector.reciprocal(rl[:, col : col + 1], o[h][:, 64:65])
                nc.vector.tensor_scalar_mul(
                    OSb[:, tt * IC + h * 64 : tt * IC + (h + 1) * 64],
                    o[h][:, 0:64], rl[:, col : col + 1],
                )

        def proj(tt):
            ptE = pstE.tile([128, 128], BF)
            nc.tensor.transpose(ptE[:], OSb[:, ts(tt, IC)], ident[:])
            otb = otbp.tile([128, 128], BF)
            nc.vector.tensor_copy(otb[:], ptE[:])
            pr = pspr.tile([128, 512], F32)
            nc.tensor.matmul(pr[:], otb[:], wp_sb, start=True, stop=True)
            ob = comb.tile([128, 512], F32, tag="ob")
            nc.vector.tensor_copy(ob[:], pr[:])
            nc.sync.dma_start(out=out_d[ts(tt, IC), :], in_=ob[:])

        for tt in range(NQC):
            attention(tt)
            if tt > 0:
                proj(tt - 1)
        proj(NQC - 1)


_PROGRAM_CACHE = {}
TRACE = False
_LAST = {}


def _build_program():
    key = ("prog", GRP)
    if key in _PROGRAM_CACHE:
        return _PROGRAM_CACHE[key]
    from contextlib import ExitStack

    nc = bacc.Bacc(trn_type="TRN2", target_bir_lowering=False, debug=False,
                   num_devices=NCORES)
    in_d = {
        "x": nc.dram_tensor("x", [T, C], BF, kind="ExternalInput").ap(),
        "blob": nc.dram_tensor("blob", [128, BLOB_W], BF, kind="ExternalInput").ap(),
        "bias": nc.dram_tensor("bias", [128, 3], F32, kind="ExternalInput").ap(),
        "ident": nc.dram_tensor("ident", [128, 128], BF, kind="ExternalInput").ap(),
        "mask_k": nc.dram_tensor("mask_k", [64, T], BF, kind="ExternalInput").ap(),
        "mask_q": nc.dram_tensor("mask_q", [64, T], BF, kind="ExternalInput").ap(),
    }
    out_d = nc.dram_tensor("out", [T, C], F32, kind="ExternalOutput").ap()
    with tile.TileContext(nc) as tc:
        with ExitStack() as ctx:
            _emit(ctx, tc, out_d, in_d)
    nc.compile()
    _PROGRAM_CACHE[key] = nc
    return nc


def _consts():
    f = np.float32
    ident = np.eye(128, dtype=f)
    rr = np.arange(64)
    jj = np.arange(T)
    # mask_k[r, j] = 1 where r == 32*(jb%2) + (j%128)//4 (jb = j//128)
    mk = (rr[:, None] == 32 * ((jj[None, :] // 128) % 2) + (jj[None, :] % 128) // NA)
    mask_k = mk.astype(f)
    # mask_q[r, i] = NEG where (i%256)//4 < r
    mask_q = np.where((jj[None, :] % 256) // NA < rr[:, None], f(NEG), f(0.0)).astype(f)
    identh = np.vstack([np.eye(64, dtype=f), np.eye(64, dtype=f)])
    return ident, mask_k, mask_q, identh


def _numpy_reference(x, mask, Wq, bq, Wk, bk, Wv, bv, Wp, bp):
    b, t, c = x.shape
    h, d = H, c // H
    scale = 1.0 / math.sqrt(d)
    tril = np.tril(np.ones((NF, NF), dtype=np.float32))
    block = np.kron(tril, np.ones((NA, NA), dtype=np.float32))

    def heads(w, bias):
        return (x @ w + bias).reshape(b, t, h, d).transpose(0, 2, 1, 3)

    q, k, v = heads(Wq, bq), heads(Wk, bk), heads(Wv, bv)
    att = np.einsum("bhqd,bhkd->bhqk", q, k) * scale
    allowed = block[None, None] * mask[:, None, None, :].astype(np.float32)
    att = np.where(allowed == 0, -np.inf, att)
    att = att - att.max(axis=-1, keepdims=True)
    att = np.exp(att)
    att = att / att.sum(axis=-1, keepdims=True)
    y = np.einsum("bhqk,bhkd->bhqd", att, v)
    y = y.transpose(0, 2, 1, 3).reshape(b, t, c)
    return (y @ Wp + bp).astype(np.float32)


def _bf(a):
    return np.ascontiguousarray(a).astype(BF_NP)


def _fold(w):
    """[512, 128] -> [128, 512] SBUF layout: out[p, cb*128+j] = w[cb*128+p, j]."""
    return w.reshape(4, 128, 128).transpose(1, 0, 2).reshape(128, 512)


def kernel(**inputs):
    x = np.asarray(inputs["x"], dtype=np.float32)
    mask = np.asarray(inputs["mask"])
    Wq = np.asarray(inputs["Wq"], dtype=np.float32)
    bq = np.asarray(inputs["bq"], dtype=np.float32)
    Wk = np.asarray(inputs["Wk"], dtype=np.float32)
    bk = np.asarray(inputs["bk"], dtype=np.float32)
    Wv = np.asarray(inputs["Wv"], dtype=np.float32)
    bv = np.asarray(inputs["bv"], dtype=np.float32)
    Wp = np.asarray(inputs["Wp"], dtype=np.float32)
    bp = np.asarray(inputs["bp"], dtype=np.float32)

    if not np.all(np.asarray(mask) == 1):
        return _numpy_reference(x, mask, Wq, bq, Wk, bk, Wv, bv, Wp, bp)

    nc = _build_program()
    ident, mask_k, mask_q, identh = _consts()
    in_maps = []
    for core in range(NCORES):
        b = core // CPB
        hp = core % CPB
        cs = slice(hp * 128, (hp + 1) * 128)
        blob = np.concatenate([
            _fold(Wq[:, cs] * np.float32(SCALE)),
            _fold(Wk[:, cs]),
            _fold(Wv[:, cs]),
            Wp[cs, :],
            identh,
        ], axis=1)
        assert blob.shape == (128, BLOB_W), blob.shape
        bias = np.stack([bq[cs] * np.float32(SCALE), bk[cs], bv[cs]],
                        axis=1).astype(np.float32)
        in_maps.append({
            "x": _bf(x[b]),
            "blob": _bf(blob),
            "bias": bias,
            "ident": _bf(ident),
            "mask_k": _bf(mask_k),
            "mask_q": _bf(mask_q),
        })
    rr = run_bass_kernel_spmd(
        nc, in_maps, list(range(NCORES)), trace=TRACE,
        tmpdir=_LAST.get("tmpdir"),
    )
    _LAST["results"] = rr
    res = rr.results
    out = np.zeros((B, T, C), dtype=np.float32)
    for b in range(B):
        acc = res[b * CPB]["out"].astype(np.float32)
        for j in range(1, CPB):
            acc = acc + res[b * CPB + j]["out"]
        out[b] = acc + bp[None, :]
    return out


# revision 15
# speedup vs baseline: 1.2477x; 1.1012x over previous
"""Block-causal multi-head self-attention for TRN2, sharded over 8 NeuronCores.

Problem (hardcoded): B=2, T=2048 (512 frames x 4 animals), C=512, H=8 heads,
D=64. Block-causal mask = kron(tril(frames), ones(4,4)); key mask is all-ones
per the input spec (a numpy fallback handles the general case).

Sharding: core i handles batch b = i//4 and heads (2*(i%4), 2*(i%4)+1).
Wq/Wk/Wv are column-sharded (128 cols per core), Wp row-sharded (128 rows per
core). Each core emits a partial [T, C] output projection; the host sums the
4 partials per batch and adds bp.

On-core algorithm (bf16 matmul operands, fp32 PSUM accumulation):
  A: x streamed in 16 row-tiles (DMA issue alternates the two HWDGE rings);
     PE-transposed into xT[c, t]. Weights arrive as one packed blob DMA.
  B: qT/kT/vT = W.T @ xT (bias folded into the PSUM->SBUF copy; softmax
     scale folded into Wq on host).
  C: V_aug blocks [128 j, 65] per (head, j-block); col 64 = ones.
  D: per 128-query chunk tt (j-blocks 0..tt; only jb==tt needs the intra-
     block tri-mask, added via rank-64 indicator rows inside the S matmul):
       S^T[j,i] groups of <=4 j-blocks x 2 heads -> one [128,<=1024] PSUM
       P^T = exp(S^T)  (ACT, PSUM -> bf16 SBUF)
       O[i, d_aug] += P^T(block).T @ V_aug(block)   (N=65 rows per matmul;
         col 64 accumulates l)
     rl = 1/O[:,64] (DVE); OS[t, (h d)] = O[:,:64]*rl (DVE, bf16)
     one PE transpose of OS[t-tile] -> stacked OT[(h d), t]; single K=128
     matmul (OT, Wp-stacked) -> partial out tile; DVE copy; DMA out.
     The projection tail of chunk tt is emitted during chunk tt+1.
"""

import math

import numpy as np
import ml_dtypes

import concourse.bass as bass
import concourse.bacc as bacc
import concourse.tile as tile
from concourse import mybir
from concourse.bass_utils import run_bass_kernel_spmd

B, T, C, H, D = 2, 2048, 512, 8, 64
NF, NA = 512, 4
NCORES = 8
HPC = 2            # heads per core
CPB = 4            # cores per batch
SCALE = 1.0 / math.sqrt(D)
NEG = -1.0e9
IC = 128           # query-chunk width
NQC = T // IC      # 16
GRP = 4            # j-blocks per exp group (per head)

F32 = mybir.dt.float32
BF = mybir.dt.bfloat16
BF_NP = ml_dtypes.bfloat16

# packed weight blob column offsets (bf16, [128, BLOB_W])
_OFF_WQ, _OFF_WK, _OFF_WV = 0, 512, 1024
_OFF_WP, _OFF_IDH = 1536, 2048
BLOB_W = 2112


def _emit(ctx, tc, out_d, in_d):
    nc = tc.nc
    ts = bass.ts
    Exp = mybir.ActivationFunctionType.Exp
    Ident = mybir.ActivationFunctionType.Identity

    const = ctx.enter_context(tc.tile_pool(name="const", bufs=1))
    big = ctx.enter_context(tc.tile_pool(name="big", bufs=1))

    # ident first on the SP ring (phase A needs it); blob + biases on the ACT
    # ring so descriptor generation for x never queues behind them.
    ident = const.tile([128, 128], BF)
    nc.sync.dma_start(out=ident[:], in_=in_d["ident"][:])
    blob = const.tile([128, BLOB_W], BF)
    nc.scalar.dma_start(out=blob[:], in_=in_d["blob"][:])
    bias = const.tile([128, 3], F32)
    nc.scalar.dma_start(out=bias[:], in_=in_d["bias"][:])

    w_sb = {"wq": blob[:, _OFF_WQ : _OFF_WQ + 512],
            "wk": blob[:, _OFF_WK : _OFF_WK + 512],
            "wv": blob[:, _OFF_WV : _OFF_WV + 512]}
    b_sb = {"wq": bias[:, 0:1], "wk": bias[:, 1:2], "wv": bias[:, 2:3]}
    wp_sb = blob[:, _OFF_WP : _OFF_WP + 512]
    identh = blob[:, _OFF_IDH : _OFF_IDH + 64]

    xT = big.tile([128, 4 * T], BF, tag="xT")
    qA = [big.tile([128, T], BF, tag=f"qA{h}", name=f"qA{h}") for h in range(HPC)]
    kA = [big.tile([128, T], BF, tag=f"kA{h}", name=f"kA{h}") for h in range(HPC)]
    vT = big.tile([128, T], BF, tag="vT")
    # mask rows: head0 tiles carry them in partitions 64:128, head1 in 0:64
    nc.sync.dma_start(out=kA[0][64:128, :], in_=in_d["mask_k"][:])
    nc.scalar.dma_start(out=kA[1][0:64, :], in_=in_d["mask_k"][:])
    nc.sync.dma_start(out=qA[0][64:128, :], in_=in_d["mask_q"][:])
    nc.scalar.dma_start(out=qA[1][0:64, :], in_=in_d["mask_q"][:])
    Vb = big.tile([128, HPC * 16 * 65], BF, tag="Vb")
    OSb = big.tile([128, T], BF, tag="OSb")
    rl = big.tile([128, HPC * NQC], F32, tag="rl")

    # Phases A/B/C interleaved per 512-token stripe t5: stream 4 x row-tiles
    # in, PE-transpose them into xT, immediately project that stripe of
    # qT/kT/vT (bias folded into the PSUM->SBUF copy), then build its V_aug
    # blocks [128 j, 65] (col 64 = ones).
    with tc.tile_pool(name="xin", bufs=4) as xin, \
         tc.tile_pool(name="pst", bufs=4, space="PSUM") as pst, \
         tc.tile_pool(name="psproj", bufs=2, space="PSUM") as psp, \
         tc.tile_pool(name="psv", bufs=2, space="PSUM") as psv:
        for t5 in range(4):
            for tb in range(4 * t5, 4 * t5 + 4):
                xt_ = xin.tile([128, 512], BF)
                eng = nc.sync if tb % 2 == 0 else nc.scalar
                eng.dma_start(out=xt_[:], in_=in_d["x"][ts(tb, 128), :])
                for cb in range(4):
                    pt = pst.tile([128, 128], BF)
                    nc.tensor.transpose(pt[:], xt_[:, ts(cb, 128)], ident[:])
                    dst = xT[:, cb * T + tb * 128 : cb * T + (tb + 1) * 128]
                    if cb % 2 == 0:
                        nc.vector.tensor_copy(dst, pt[:])
                    else:
                        nc.scalar.activation(dst, pt[:], Ident)
            for nm, dst in (("wq", qA), ("wk", kA), ("wv", vT)):
                ps = psp.tile([128, 512], F32)
                for cb in range(4):
                    nc.tensor.matmul(
                        ps[:],
                        w_sb[nm][:, ts(cb, 128)],
                        xT[:, cb * T + t5 * 512 : cb * T + (t5 + 1) * 512],
                        start=(cb == 0),
                        stop=(cb == 3),
                    )
                if nm == "wv":
                    if t5 % 2 == 0:
                        nc.scalar.activation(dst[:, ts(t5, 512)], ps[:], Ident,
                                             bias=b_sb[nm])
                    else:
                        nc.vector.tensor_scalar_add(dst[:, ts(t5, 512)], ps[:],
                                                    b_sb[nm])
                else:
                    nc.scalar.activation(dst[0][0:64, ts(t5, 512)], ps[0:64, :],
                                         Ident, bias=b_sb[nm][0:64, :])
                    nc.vector.tensor_scalar_add(dst[1][64:128, ts(t5, 512)],
                                                ps[64:128, :],
                                                b_sb[nm][64:128, :])
            for h in range(HPC):
                for jb in range(4 * t5, 4 * t5 + 4):
                    pv = psv.tile([128, 64], BF)
                    nc.tensor.transpose(
                        pv[:], vT[ts(h, 64), ts(jb, 128)], identh[ts(h, 64), :]
                    )
                    base = (h * 16 + jb) * 65
                    nc.vector.tensor_copy(Vb[:, base : base + 64], pv[:])
                    nc.vector.memset(Vb[:, base + 64 : base + 65], 1.0)

    # Phase D: attention (128-query chunks) + projection, software-pipelined.
    with tc.tile_pool(name="pss", bufs=2, space="PSUM") as pss, \
         tc.tile_pool(name="pso", bufs=1, space="PSUM") as pso, \
         tc.tile_pool(name="pstE", bufs=1, space="PSUM") as pstE, \
         tc.tile_pool(name="pspr", bufs=1, space="PSUM") as pspr, \
         tc.tile_pool(name="ptp", bufs=3) as ptp, \
         tc.tile_pool(name="otbp", bufs=2) as otbp, \
         tc.tile_pool(name="comb", bufs=2) as comb:

        def attention(tt):
            nbj = tt + 1
            o = [pso.tile([128, 65], F32, tag=f"o{h}", name=f"o{h}")
                 for h in range(HPC)]
            for g0 in range(0, nbj, GRP):
                ng = min(GRP, nbj - g0)
                ps = pss.tile([128, 2 * GRP * 128], F32)
                for h in range(HPC):
                    hs = slice(0, 64) if h == 0 else slice(64, 128)
                    for k in range(ng):
                        jb = g0 + k
                        sl = ps[:, (h * ng + k) * 128 : (h * ng + k + 1) * 128]
                        if jb == tt:
                            nc.tensor.matmul(
                                sl, kA[h][:, ts(jb, 128)], qA[h][:, ts(tt, IC)],
                                start=True, stop=True,
                            )
                        else:
                            nc.tensor.matmul(
                                sl, kA[h][hs, ts(jb, 128)], qA[h][hs, ts(tt, IC)],
                                start=True, stop=True,
                            )
                ptt = ptp.tile([128, 2 * GRP * 128], BF)
                nc.scalar.activation(ptt[:, 0 : 2 * ng * 128],
                                     ps[:, 0 : 2 * ng * 128], Exp)
                for h in range(HPC):
                    for k in range(ng):
                        jb = g0 + k
                        vbase = (h * 16 + jb) * 65
                        nc.tensor.matmul(
                            o[h][:],
                            ptt[:, (h * ng + k) * 128 : (h * ng + k + 1) * 128],
                            Vb[:, vbase : vbase + 65],
                            start=(jb == 0),
                            stop=(jb == nbj - 1),
                        )
            for h in range(HPC):
                col = h * NQC + tt
                nc.vector.reciprocal(rl[:, col : col + 1], o[h][:, 64:65])
                nc.vector.tensor_scalar_mul(
                    OSb[:, tt * IC + h * 64 : tt * IC + (h + 1) * 64],
                    o[h][:, 0:64], rl[:, col : col + 1],
                )

        def proj(tt):
            ptE = pstE.tile([128, 128], BF)
            nc.tensor.transpose(ptE[:], OSb[:, ts(tt, IC)], ident[:])
            otb = otbp.tile([128, 128], BF)
            nc.vector.tensor_copy(otb[:], ptE[:])
            pr = pspr.tile([128, 512], F32)
            nc.tensor.matmul(pr[:], otb[:], wp_sb, start=True, stop=True)
            ob = comb.tile([128, 512], F32, tag="ob")
            nc.vector.tensor_copy(ob[:], pr[:])
            nc.sync.dma_start(out=out_d[ts(tt, IC), :], in_=ob[:])

        for tt in range(NQC):
            attention(tt)
            if tt > 0:
                proj(tt - 1)
        proj(NQC - 1)


_PROGRAM_CACHE = {}
TRACE = False
_LAST = {}


def _build_program():
    key = ("prog", GRP)
    if key in _PROGRAM_CACHE:
        return _PROGRAM_CACHE[key]
    from contextlib import ExitStack

    nc = bacc.Bacc(trn_type="TRN2", target_bir_lowering=False, debug=False,
                   num_devices=NCORES)
    in_d = {
        "x": nc.dram_tensor("x", [T, C], BF, kind="ExternalInput").ap(),
        "blob": nc.dram_tensor("blob", [128, BLOB_W], BF, kind="ExternalInput").ap(),
        "bias": nc.dram_tensor("bias", [128, 3], F32, kind="ExternalInput").ap(),
        "ident": nc.dram_tensor("ident", [128, 128], BF, kind="ExternalInput").ap(),
        "mask_k": nc.dram_tensor("mask_k", [64, T], BF, kind="ExternalInput").ap(),
        "mask_q": nc.dram_tensor("mask_q", [64, T], BF, kind="ExternalInput").ap(),
    }
    out_d = nc.dram_tensor("out", [T, C], F32, kind="ExternalOutput").ap()
    with tile.TileContext(nc) as tc:
        with ExitStack() as ctx:
            _emit(ctx, tc, out_d, in_d)
    nc.compile()
    _PROGRAM_CACHE[key] = nc
    return nc


def _consts():
    f = np.float32
    ident = np.eye(128, dtype=f)
    rr = np.arange(64)
    jj = np.arange(T)
    # mask_k[r, j] = 1 where r == 32*(jb%2) + (j%128)//4 (jb = j//128)
    mk = (rr[:, None] == 32 * ((jj[None, :] // 128) % 2) + (jj[None, :] % 128) // NA)
    mask_k = mk.astype(f)
    # mask_q[r, i] = NEG where (i%256)//4 < r
    mask_q = np.where((jj[None, :] % 256) // NA < rr[:, None], f(NEG), f(0.0)).astype(f)
    identh = np.vstack([np.eye(64, dtype=f), np.eye(64, dtype=f)])
    return ident, mask_k, mask_q, identh


def _numpy_reference(x, mask, Wq, bq, Wk, bk, Wv, bv, Wp, bp):
    b, t, c = x.shape
    h, d = H, c // H
    scale = 1.0 / math.sqrt(d)
    tril = np.tril(np.ones((NF, NF), dtype=np.float32))
    block = np.kron(tril, np.ones((NA, NA), dtype=np.float32))

    def heads(w, bias):
        return (x @ w + bias).reshape(b, t, h, d).transpose(0, 2, 1, 3)

    q, k, v = heads(Wq, bq), heads(Wk, bk), heads(Wv, bv)
    att = np.einsum("bhqd,bhkd->bhqk", q, k) * scale
    allowed = block[None, None] * mask[:, None, None, :].astype(np.float32)
    att = np.where(allowed == 0, -np.inf, att)
    att = att - att.max(axis=-1, keepdims=True)
    att = np.exp(att)
    att = att / att.sum(axis=-1, keepdims=True)
    y = np.einsum("bhqk,bhkd->bhqd", att, v)
    y = y.transpose(0, 2, 1, 3).reshape(b, t, c)
    return (y @ Wp + bp).astype(np.float32)


def _bf(a):
    return np.ascontiguousarray(a).astype(BF_NP)


def _fold(w):
    """[512, 128] -> [128, 512] SBUF layout: out[p, cb*128+j] = w[cb*128+p, j]."""
    return w.reshape(4, 128, 128).transpose(1, 0, 2).reshape(128, 512)


def kernel(**inputs):
    x = np.asarray(inputs["x"], dtype=np.float32)
    mask = np.asarray(inputs["mask"])
    Wq = np.asarray(inputs["Wq"], dtype=np.float32)
    bq = np.asarray(inputs["bq"], dtype=np.float32)
    Wk = np.asarray(inputs["Wk"], dtype=np.float32)
    bk = np.asarray(inputs["bk"], dtype=np.float32)
    Wv = np.asarray(inputs["Wv"], dtype=np.float32)
    bv = np.asarray(inputs["bv"], dtype=np.float32)
    Wp = np.asarray(inputs["Wp"], dtype=np.float32)
    bp = np.asarray(inputs["bp"], dtype=np.float32)

    if not np.all(np.asarray(mask) == 1):
        return _numpy_reference(x, mask, Wq, bq, Wk, bk, Wv, bv, Wp, bp)

    nc = _build_program()
    ident, mask_k, mask_q, identh = _consts()
    in_maps = []
    for core in range(NCORES):
        b = core // CPB
        hp = core % CPB
        cs = slice(hp * 128, (hp + 1) * 128)
        blob = np.concatenate([
            _fold(Wq[:, cs] * np.float32(SCALE)),
            _fold(Wk[:, cs]),
            _fold(Wv[:, cs]),
            Wp[cs, :],
            identh,
        ], axis=1)
        assert blob.shape == (128, BLOB_W), blob.shape
        bias = np.stack([bq[cs] * np.float32(SCALE), bk[cs], bv[cs]],
                        axis=1).astype(np.float32)
        in_maps.append({
            "x": _bf(x[b]),
            "blob": _bf(blob),
            "bias": bias,
            "ident": _bf(ident),
            "mask_k": _bf(mask_k),
            "mask_q": _bf(mask_q),
        })
    rr = run_bass_kernel_spmd(
        nc, in_maps, list(range(NCORES)), trace=TRACE,
        tmpdir=_LAST.get("tmpdir"),
    )
    _LAST["results"] = rr
    res = rr.results
    out = np.zeros((B, T, C), dtype=np.float32)
    for b in range(B):
        acc = res[b * CPB]["out"].astype(np.float32)
        for j in range(1, CPB):
            acc = acc + res[b * CPB + j]["out"]
        out[b] = acc + bp[None, :]
    return out
